# revision 60
# baseline (speedup 1.0000x reference)
"""DualAttention Trainium2 kernel: 8-core data-parallel over batch.

Each NeuronCore processes one batch element [1024, 512] end to end:
q/k/v projections, spatial+channel gated value, 8-head attention, output
projection. All matmuls run in float32r (TF32-like) at 1 cycle/row on
the PE. Activations are kept "transposed" (d on partitions, tokens on
the free dim) so every matmul contracts over the partition dim;
attention scores are computed transposed ([k_tok, q_tok]) so the PV
matmul needs no on-chip transpose of the probability matrix. The
softmax denominator comes for free from a ones-column appended to V in
the PV matmul (row 64 of the PSUM accumulator); normalization is a
partition-broadcast of the sums + one divide per head.

Phase order is chosen for engine overlap: the value/gating path runs
first (ACT idle anyway), q/k projections next, then the attention
phase streams scores->exp->PV per 128-token chunk with a 2-chunk lag
so the Activation engine (the attention-phase pacer) never starves.
"""
import numpy as np

import concourse.bass as bass
import concourse.tile as tile
from concourse import bacc, mybir
from concourse.bass_utils import run_bass_kernel_spmd
from concourse.masks import make_identity

B, N, D = 8, 1024, 512
H, DH, HID = 8, 64, 256
P = 128
K4 = D // P          # 4 d-chunks
T8 = N // P          # 8 token chunks
M2 = HID // P        # 2 hidden chunks
NCORES = 8
PV_LAG = 4

F32 = mybir.dt.float32
F32R = mybir.dt.float32r
AF = mybir.ActivationFunctionType
OP = mybir.AluOpType
AX = mybir.AxisListType

# config switches (fallbacks for compiler restrictions)
W_DIRECT = True      # DMA weights straight into f32r tiles (no DVE round copy)
USE_DIVIDE = True    # normalize via tensor_tensor divide (else recip+mult)

WEIGHT_NAMES = ["Wq", "Wk", "Wv", "Wo", "Ws1", "Ws2", "Wc1", "Wc2"]
BIAS_NAMES = ["bq", "bk", "bv", "bo", "bs1", "bs2", "bc1", "bc2"]

_CACHE = {}


def _build():
    nc = bacc.Bacc("TRN2", target_bir_lowering=False)

    wdt = F32R if W_DIRECT else F32
    query_h = nc.dram_tensor("query", [N, D], F32, kind="ExternalInput")
    key_h = nc.dram_tensor("key_in", [N, D], F32, kind="ExternalInput")
    value_h = nc.dram_tensor("value", [N, D], F32, kind="ExternalInput")
    wshape = {"Wq": [D, D], "Wk": [D, D], "Wv": [D, D], "Wo": [D, D],
              "Ws1": [D, HID], "Ws2": [HID, D],
              "Wc1": [D, HID], "Wc2": [HID, D]}
    w_h = {nm: nc.dram_tensor(nm, wshape[nm],
                              F32 if nm in ("Wc1", "Wc2") else wdt,
                              kind="ExternalInput")
           for nm in WEIGHT_NAMES}
    b_h = {
        nm: nc.dram_tensor(nm, [HID if nm in ("bs1", "bc1") else D], F32,
                           kind="ExternalInput")
        for nm in BIAS_NAMES
    }
    out_h = nc.dram_tensor("out", [N, D], F32, kind="ExternalOutput")

    with tile.TileContext(nc) as tc:
        with tc.tile_pool(name="const", bufs=1) as cpool, \
             tc.tile_pool(name="wrest", bufs=1) as wrest, \
             tc.tile_pool(name="big", bufs=1) as big, \
             tc.tile_pool(name="xst", bufs=4) as xst, \
             tc.tile_pool(name="osb", bufs=4) as osp:
            psA = tc.alloc_tile_pool(name="psA", bufs=4, space="PSUM",
                                     side="left")

            id_t = cpool.tile([P, P], F32, tag="ident")
            make_identity(nc, id_t[:])

            def load_weight(name, pool, rows, cols, dt=F32R):
                nk = rows // P
                wt = pool.tile([P, nk * cols], dt, tag=name)
                for k in range(nk):
                    nc.sync.dma_start(wt[:, k * cols:(k + 1) * cols],
                                      w_h[name][k * P:(k + 1) * P, :])
                return [wt[:, k * cols:(k + 1) * cols] for k in range(nk)]

            def load_weight_mcols(name, pool, ms):
                # column-sliced load of a [D, D] weight: only m-chunks in
                # `ms` are transferred (64KB per (m, k) piece). Lets the
                # m0 slice land early while m1-3 ride the idle DMA stream
                # during attention.
                wt = pool.tile([P, K4 * D], F32R, tag=name)
                for m in ms:
                    for k in range(K4):
                        nc.sync.dma_start(
                            wt[:, k * D + m * P:k * D + (m + 1) * P],
                            w_h[name][k * P:(k + 1) * P, m * P:(m + 1) * P])
                return [wt[:, k * D:(k + 1) * D] for k in range(K4)]

            def load_bias(name, rows):
                # one strided DMA: bt[p, k] = bias[k*128 + p]
                nk = rows // P
                bt = cpool.tile([P, nk], F32, tag=name)
                nc.sync.dma_start(
                    bt[:], b_h[name][:].rearrange("(k p) -> p k", p=P))
                return [bt[:, k:k + 1] for k in range(nk)]

            # persistent activations, transposed layout [128, 4*1024]:
            # d-chunk k at columns k*1024.., token index within
            qT = big.tile([P, K4 * N], F32R, tag="qT")
            kT = big.tile([P, K4 * N], F32R, tag="kT")
            vvT = big.tile([P, K4 * N], F32R, tag="vvT")   # vv -> Vs -> Vd
            vdE = big.tile([P, T8 * H * (DH + 1)], F32R, tag="vdE")
            outT = big.tile([P, K4 * N], F32R, tag="outT")
            vtp = tc.alloc_tile_pool(name="vtp", bufs=1)
            s1p = tc.alloc_tile_pool(name="s1p", bufs=1)
            vT_in = vtp.tile([P, K4 * N], F32R, tag="vT_in")
            s1T = s1p.tile([P, M2 * N], F32R, tag="s1T")

            def transpose_pair(src_dram, t8, dst_views, psum_pool=None,
                               copy_eng=None):
                pool = psum_pool or psA
                for j in range(2):
                    st = xst.tile([P, D], F32, tag="xst")
                    nc.sync.dma_start(st[:],
                                      src_dram[(t8 + j) * P:(t8 + j + 1) * P, :])
                    ps = pool.tile([P, D], F32,
                                   tag="psA" if pool is psA else "psP")
                    for k in range(K4):
                        nc.tensor.transpose(ps[:, k * P:(k + 1) * P],
                                            st[:, k * P:(k + 1) * P], id_t[:])
                    src = ps[:].rearrange("p (k t) -> p k t", k=K4)
                    # ACT is idle through the input phase: copy there, not
                    # DVE — but during attention ACT is the pacer, use DVE
                    if copy_eng == "dve":
                        nc.vector.tensor_copy(dst_views[j], src)
                    else:
                        nc.scalar.copy(dst_views[j], src)

            hpool = tc.alloc_tile_pool(name="half", bufs=4, side="right")
            swp = tc.alloc_tile_pool(name="swt", bufs=2)

            # ---------------- value path + gating ----------------
            # fine-grained interleave: PE rides just behind the DMA input
            # stream; independent transposes fill chain-latency bubbles.
            extv = vdE[:].rearrange("p (t h c) -> p t h c", t=T8, h=H)
            ones64 = cpool.tile([P, T8 * H], F32, tag="ones64")
            nc.vector.memset(ones64[:], 1.0)

            def v_tr2(t8):
                vw = vT_in[:].rearrange("p (k t) -> p k t", k=K4)
                transpose_pair(value_h, t8, [
                    vw[:, :, (t8 + j) * P:(t8 + j + 1) * P] for j in range(2)])

            def vv_group(m, half):
                ps = psA.tile([P, 512], F32, tag="psA")
                for k in range(K4):
                    nc.tensor.matmul(
                        ps[:], wv_t[k][:, m * P:(m + 1) * P],
                        vT_in[:, k * N + half * 512:k * N + half * 512 + 512],
                        start=(k == 0), stop=(k == K4 - 1))
                c0 = m * N + half * 512
                nc.scalar.add(out=vvT[:, c0:c0 + 512], in_=ps[:],
                              add=bv_t[m][:])

            def s1_group(m, half):
                ps = psA.tile([P, 512], F32, tag="psA")
                for k in range(K4):
                    nc.tensor.matmul(
                        ps[:], ws1_t[k][:, m * P:(m + 1) * P],
                        vvT[:, k * N + half * 512:k * N + half * 512 + 512],
                        start=(k == 0), stop=(k == K4 - 1))
                c0 = m * N + half * 512
                nc.scalar.activation(s1T[:, c0:c0 + 512], ps[:], AF.Relu,
                                     bias=bs1_t[m][:])

            def sw_group(m, half):
                ps = psA.tile([P, 512], F32, tag="psA")
                for k in range(M2):
                    nc.tensor.matmul(
                        ps[:], ws2_t[k][:, m * P:(m + 1) * P],
                        s1T[:, k * N + half * 512:k * N + half * 512 + 512],
                        start=(k == 0), stop=(k == M2 - 1))
                sw = swp.tile([P, 512], F32, tag="swt")
                nc.scalar.activation(sw[:], ps[:], AF.Sigmoid,
                                     bias=bs2_t[m][:])
                sl = slice(m * N + half * 512, m * N + half * 512 + 512)
                nc.vector.tensor_tensor(out=vvT[:, sl], in0=vvT[:, sl],
                                        in1=sw[:], op=OP.mult)

            def vch_group(m, half, psum_pool=None):
                pool = psum_pool or psA
                ps = pool.tile([P, 512], F32,
                               tag="psA" if pool is psA else "psP")
                for k in range(K4):
                    nc.tensor.matmul(
                        ps[:], wv_t[k][:, m * P:(m + 1) * P],
                        vT_in[:, k * N + half * 512:k * N + half * 512 + 512],
                        start=(k == 0), stop=(k == K4 - 1))
                sl = slice(m * N + half * 512, m * N + half * 512 + 512)
                nc.vector.scalar_tensor_tensor(
                    out=vvT[:, sl], in0=ps[:], scalar=bv_t[m][:],
                    in1=vvT[:, sl], op0=OP.add, op1=OP.add)

            def vd_tr(t8, psum_pool=None, copy_eng=None):
                pool = psum_pool or psA
                ps = pool.tile([P, D], F32,
                               tag="psA" if pool is psA else "psP")
                for k in range(K4):
                    nc.tensor.transpose(
                        ps[:, k * P:(k + 1) * P],
                        vvT[:, k * N + t8 * P:k * N + (t8 + 1) * P].bitcast(F32),
                        id_t[:])
                src = ps[:].rearrange("p (h c) -> p h c", h=H)
                if copy_eng == "dve":
                    nc.vector.tensor_copy(extv[:, t8, :, 0:DH], src)
                else:
                    nc.scalar.copy(extv[:, t8, :, 0:DH], src)

            def x_tr2(src, ht, half, i, psum_pool=None, copy_eng=None):
                hw = ht[:].rearrange("p (k t) -> p k t", k=K4)
                transpose_pair(src, half * 4 + i, [
                    hw[:, :, (i + j) * P:(i + j + 1) * P] for j in range(2)],
                    psum_pool=psum_pool, copy_eng=copy_eng)

            # value half0 -> vv half0 (starts after only 4 input chunks)
            for t8 in range(0, 4, 2):
                v_tr2(t8)
            wv_t = load_weight("Wv", wrest, D, D)
            bv_t = load_bias("bv", D)
            for m in range(K4):
                vv_group(m, 0)
            for t8 in range(4, T8, 2):
                v_tr2(t8)
            wc1_t = load_weight("Wc1", wrest, D, HID, dt=F32)
            bc1_t = load_bias("bc1", HID)
            for m in range(K4):
                vv_group(m, 1)

            # channel profile MLP stage 1 (prof needs the full vT_in)
            profr = []
            for k in range(K4):
                pr = cpool.tile([P, 1], F32, tag=f"prof{k}")
                nc.vector.reduce_sum(pr[:], vT_in[:, k * N:(k + 1) * N],
                                     axis=AX.X)
                prr = cpool.tile([P, 1], F32, tag=f"profr{k}")
                nc.vector.tensor_scalar_mul(prr[:], pr[:], 1.0 / N)
                profr.append(prr)
            ws1_t = load_weight("Ws1", wrest, D, HID)
            bs1_t = load_bias("bs1", HID)
            c1r = []
            for m in range(M2):
                ps = psA.tile([P, 512], F32, tag="psA")
                for k in range(K4):
                    nc.tensor.matmul(
                        ps[:, 0:1], wc1_t[k][:, m * P:(m + 1) * P],
                        profr[k][:], start=(k == 0), stop=(k == K4 - 1))
                cr = cpool.tile([P, 1], F32, tag=f"c1r{m}")
                nc.scalar.activation(cr[:], ps[:, 0:1], AF.Relu,
                                     bias=bc1_t[m][:])
                c1r.append(cr)

            # s1 half0, then query transposes stream in (DMA-paced filler)
            for m in range(M2):
                s1_group(m, 0)
            qh0 = hpool.tile([P, K4 * 512], F32R, tag="half")
            qh1 = hpool.tile([P, K4 * 512], F32R, tag="half")
            q_halves = [qh0, qh1]
            # interleave: q transposes + s1 half1
            x_tr2(query_h, q_halves[0], 0, 0)
            s1_group(0, 1)
            s1_group(1, 1)
            x_tr2(query_h, q_halves[0], 0, 2)

            # channel MLP stage 2 + row scale of vT_in by cw
            wc2_t = load_weight("Wc2", wrest, HID, D, dt=F32)
            bc2_t = load_bias("bc2", D)
            cw_t = []
            for m in range(K4):
                ps = psA.tile([P, 512], F32, tag="psA")
                for k in range(M2):
                    nc.tensor.matmul(
                        ps[:, 0:1], wc2_t[k][:, m * P:(m + 1) * P],
                        c1r[k][:], start=(k == 0), stop=(k == M2 - 1))
                cw = cpool.tile([P, 1], F32, tag=f"cw{m}")
                nc.scalar.activation(cw[:], ps[:, 0:1], AF.Sigmoid,
                                     bias=bc2_t[m][:])
                cw_t.append(cw)
            ws2_t = load_weight("Ws2", wrest, HID, D)
            bs2_t = load_bias("bs2", D)
            for i in range(0, 4, 2):
                x_tr2(query_h, q_halves[1], 1, i)
            for k in range(K4):
                nc.vector.tensor_scalar_mul(
                    out=vT_in[:, k * N:(k + 1) * N],
                    in0=vT_in[:, k * N:(k + 1) * N], scalar1=cw_t[k][:])

            # per half: sw/Vs -> vch/Vd -> key transposes -> Vd transpose;
            # Wq/Wk queue between the key halves so head-0 scores can start
            # from key half0 while half1 still streams in
            kh0 = hpool.tile([P, K4 * 512], F32R, tag="half")
            kh1 = hpool.tile([P, K4 * 512], F32R, tag="half")
            k_halves = [kh0, kh1]
            for half in range(2):
                for m in range(K4):
                    sw_group(m, half)
                x_tr2(key_h, k_halves[half], half, 0)
                for m in range(K4):
                    vch_group(m, half)
                x_tr2(key_h, k_halves[half], half, 2)
                for t8 in range(half * 4, half * 4 + 4):
                    vd_tr(t8)
            nc.vector.tensor_copy(
                extv[:, :, :, DH:DH + 1],
                ones64[:].rearrange("p (t h) -> p t h", t=T8)[:, :, :, None])

            # ---------------- q/k projections + attention ----------------
            wq_t = load_weight_mcols("Wq", wrest, [0])
            bq_t = load_bias("bq", D)
            wk_t = load_weight_mcols("Wk", wrest, [0])
            bk_t = load_bias("bk", D)

            def proj_mh(w_tiles, bias_tiles, halves, OUT, m, half,
                        pool=None):
                pool = pool or psP
                ps = pool.tile([P, 512], F32, tag="psP")
                for k in range(K4):
                    nc.tensor.matmul(
                        ps[:],
                        w_tiles[k][:, m * P:(m + 1) * P],
                        halves[half][:, k * 512:(k + 1) * 512],
                        start=(k == 0), stop=(k == K4 - 1))
                c0 = m * N + half * 512
                nc.vector.tensor_scalar_add(
                    out=OUT[:, c0:c0 + 512], in0=ps[:],
                    scalar1=bias_tiles[m][:])

            def proj_m(w_tiles, bias_tiles, halves, OUT, m):
                for half in range(2):
                    ps = psP.tile([P, 512], F32, tag="psP")
                    for k in range(K4):
                        nc.tensor.matmul(
                            ps[:],
                            w_tiles[k][:, m * P:(m + 1) * P],
                            halves[half][:, k * 512:(k + 1) * 512],
                            start=(k == 0), stop=(k == K4 - 1))
                    c0 = m * N + half * 512
                    nc.vector.tensor_scalar_add(
                        out=OUT[:, c0:c0 + 512], in0=ps[:],
                        scalar1=bias_tiles[m][:])

            # gating pools done; attention takes all PSUM banks
            swp.release()
            s1p.release()
            vtp.release()
            psA.release()
            psS = tc.alloc_tile_pool(name="psS", bufs=2, space="PSUM",
                                     side="right")
            psG = tc.alloc_tile_pool(name="psG", bufs=1, space="PSUM",
                                     side="left")
            # dedicated bank pair for projection/final fillers so they never
            # steal a psS slot from the scores->exp stream
            psP = tc.alloc_tile_pool(name="psP", bufs=2, space="PSUM",
                                     side="left")
            ptp = tc.alloc_tile_pool(name="ptp", bufs=5)
            rcp = tc.alloc_tile_pool(name="rcp", bufs=1)

            def scores_chunk(h, k8):
                p0 = (h % 2) * DH
                cc = (h // 2) * N
                ps = psS.tile([P, N], F32, tag="psS")
                for half in range(2):
                    nc.tensor.matmul(
                        ps[:, half * 512:(half + 1) * 512],
                        kT[p0:p0 + DH, cc + k8 * P:cc + (k8 + 1) * P],
                        qT[p0:p0 + DH,
                           cc + half * 512:cc + (half + 1) * 512],
                        start=True, stop=True)
                pt = ptp.tile([P, N], F32R, tag="pt")
                nc.scalar.activation(pt[:], ps[:], AF.Exp, scale=0.125)
                return pt

            def pv_chunk(h, k8, G, pt):
                for half in range(2):
                    nc.tensor.matmul(
                        G[:, half * 512:(half + 1) * 512],
                        extv[:, k8, h, :],
                        pt[:, half * 512:(half + 1) * 512],
                        start=(k8 == 0), stop=(k8 == T8 - 1),
                        skip_group_check=True)

            def norm_head(h, G):
                # gpsimd cannot read PSUM: reciprocal (DVE, psum->sbuf row)
                # first, then partition-broadcast, then multiply. The last
                # head multiplies in q8-column slices so the final-projection
                # k3 matmuls pipeline behind it instead of waiting for the
                # whole row.
                p0 = (h % 2) * DH
                cc = (h // 2) * N
                rec = rcp.tile([1, N], F32, tag="rec")
                nc.vector.reciprocal(rec[:], G[DH:DH + 1, :])
                recB = rcp.tile([DH, N], F32, tag="recB")
                nc.gpsimd.partition_broadcast(recB[:], rec[:])
                nc.vector.tensor_tensor(
                    out=outT[p0:p0 + DH, cc:cc + N],
                    in0=G[0:DH, :], in1=recB[:], op=OP.mult)

            def run_head(h, G, fillers=None):
                pts = []
                for k8 in range(T8):
                    pts.append(scores_chunk(h, k8))
                    if k8 >= PV_LAG:
                        pv_chunk(h, k8 - PV_LAG, G, pts[k8 - PV_LAG])
                    if fillers and k8 in fillers:
                        fillers[k8]()
                for k8 in range(T8 - PV_LAG, T8):
                    pv_chunk(h, k8, G, pts[k8])
                norm_head(h, G)

            # final-projection partials (k=0..2 + bias) run as PE fillers
            # inside heads 6/7; only the k=3 matmul stays in the tail
            accp = None
            acc_t = []

            def final_partial(q8):
                ps = psP.tile([P, 512], F32, tag="psP")
                for k in range(K4 - 1):
                    nc.tensor.matmul(
                        ps[:],
                        outT[:, k * N + q8 * P:k * N + (q8 + 1) * P],
                        wo_t[k][:],
                        start=(k == 0), stop=(k == K4 - 2))
                at = accp.tile([P, D], F32, tag=f"acc{q8}")
                nc.vector.tensor_tensor(out=at[:], in0=ps[:], in1=boB[:],
                                        op=OP.add)
                acc_t.append(at)

            proj_m(wq_t, bq_t, q_halves, qT, 0)
            proj_m(wk_t, bk_t, k_halves, kT, 0)
            # m1-3 column slices ride the now-idle DMA stream
            load_weight_mcols("Wq", wrest, [1, 2, 3])
            load_weight_mcols("Wk", wrest, [1, 2, 3])
            wo_t = load_weight("Wo", wrest, D, D)
            boB = cpool.tile([P, D], F32, tag="boB")
            nc.sync.dma_start(boB[:], b_h["bo"][None, :].to_broadcast((P, D)))
            for m in range(K4):
                fillers0 = {}
                G = psG.tile([DH + 1, N], F32, tag="G")
                run_head(2 * m, G, fillers0)
                fillers = {}
                if m < K4 - 1:
                    fillers[3] = (lambda mm=m: proj_m(
                        wq_t, bq_t, q_halves, qT, mm + 1))
                    fillers[6] = (lambda mm=m: proj_m(
                        wk_t, bk_t, k_halves, kT, mm + 1))
                elif m == K4 - 1:
                    fillers[1] = lambda: final_partial(0)
                    fillers[3] = lambda: final_partial(1)
                    fillers[5] = lambda: final_partial(2)
                    fillers[7] = lambda: final_partial(3)
                G = psG.tile([DH + 1, N], F32, tag="G")
                run_head(2 * m + 1, G, fillers)
                if m == K4 - 2:
                    hpool.release()
                    accp = tc.alloc_tile_pool(name="accp", bufs=1,
                                              side="right")
            # remaining final partials during head 7's tail window
            for q8 in range(4, T8):
                final_partial(q8)

            psP.release()
            psG.release()
            rcp.release()
            ptp.release()
            psF = tc.alloc_tile_pool(name="psF", bufs=4, space="PSUM",
                                     side="left")

            # ---- final projection tail: out = acc + outT_3.T @ Wo_3
            for q8 in range(T8):
                ps = psF.tile([P, D], F32, tag="psF")
                nc.tensor.matmul(
                    ps[:],
                    outT[:, 3 * N + q8 * P:3 * N + (q8 + 1) * P],
                    wo_t[3][:], start=True, stop=True)
                ob = osp.tile([P, D], F32, tag="osb")
                nc.vector.tensor_tensor(out=ob[:], in0=ps[:],
                                        in1=acc_t[q8][:], op=OP.add)
                nc.sync.dma_start(out_h[q8 * P:(q8 + 1) * P, :], ob[:])
            psF.release()
            psS.release()
            accp.release()

    nc.finalize()
    return nc


def get_nc():
    if "nc" not in _CACHE:
        _CACHE["nc"] = _build()
    return _CACHE["nc"]


def kernel(**inputs):
    if "key_in" not in inputs and "key" in inputs:
        inputs["key_in"] = inputs.pop("key")
    nc = get_nc()
    shared = {}
    for nm in WEIGHT_NAMES + BIAS_NAMES:
        shared[nm] = np.ascontiguousarray(np.asarray(inputs[nm], np.float32))
    in_maps = []
    for c in range(NCORES):
        m = dict(shared)
        m["query"] = np.ascontiguousarray(
            np.asarray(inputs["query"][c], np.float32))
        m["key_in"] = np.ascontiguousarray(
            np.asarray(inputs["key_in"][c], np.float32))
        m["value"] = np.ascontiguousarray(
            np.asarray(inputs["value"][c], np.float32))
        in_maps.append(m)
    res = run_bass_kernel_spmd(nc, in_maps, core_ids=list(range(NCORES)))
    return np.stack([res.results[c]["out"] for c in range(NCORES)], axis=0)
